# revision 6
# baseline (speedup 1.0000x reference)
"""Trainium2 Bass kernel for nn_BiLSTM_45612552684163.

Replicated BiLSTM + MLP on all 8 cores; pairwise stage sharded by receptor
rows (64 per core).  Performance structure vs the naive mapping:

  1. fwd/bwd LSTM cells merged into shared elementwise ops per (t, layer):
     gates psum tile [128, 32] cols (m, d, s); one 24-col sigmoid, one 8-col
     tanh, shared tensor_tensor chain for c; per-direction h writes run on
     GpSimd to offload DVE.
  2. Recurrent matmuls keep Whh stationary in bf16 (fast 32ns weight load;
     fp8 DoubleRow measured 4x slower loads on this HW and was reverted).
  3. r-branch MLP computes only this core's 64 receptor rows.
  4. Pairwise h3 = relu(pl + pr) generation split DVE/ACT; contraction
     against wout reduced to the single logit-difference column.
"""

import sys

sys.path.insert(0, "/opt/trn_rl_repo")

from contextlib import ExitStack

import numpy as np
import ml_dtypes

import concourse.bass as bass
import concourse.mybir as mybir
import concourse.tile as tile
from concourse import bacc
from concourse.bass_utils import run_bass_kernel_spmd

T = 512          # sequence length (N_R == N_L == 512)
DIN = 20
H = 250          # LSTM hidden per direction
HP = 256         # padded hidden
G4 = 4 * HP      # 1024 padded gates
H1, H2, H3, RRI = 1024, 512, 512, 2
NCORES = 8
RPC = T // NCORES  # 64 receptor rows per core

F32 = mybir.dt.float32
BF16 = mybir.dt.bfloat16
AF = mybir.ActivationFunctionType
ALU = mybir.AluOpType

_BF = ml_dtypes.bfloat16


# ----------------------------------------------------------------------------
# Host-side weight preparation
# ----------------------------------------------------------------------------

def _pad_reorder_rows(w):
    """[1000, ...] pytorch gate order (i,f,g,o) -> [1024, ...] order (i,f,o,g),
    each gate padded 250->256 with zeros."""
    i, f, g, o = w[0:250], w[250:500], w[500:750], w[750:1000]
    z = np.zeros((6,) + w.shape[1:], w.dtype)
    return np.concatenate([i, z, f, z, o, z, g, z], axis=0)


def _pad_cols_500(w):
    """[..., 500] (fwd 250 | bwd 250) -> [..., 512] (fwd 256 | bwd 256)."""
    zf = np.zeros(w.shape[:-1] + (6,), w.dtype)
    return np.concatenate([w[..., 0:250], zf, w[..., 250:500], zf], axis=-1)


def _chunk_bias(b):
    """[M] -> [128, M//128] per-partition bias layout (col m = chunk m)."""
    return np.ascontiguousarray(b.reshape(-1, 128).T)


def _prep_inputs(inp):
    bf = lambda a: np.ascontiguousarray(np.asarray(a, np.float32)).astype(_BF)
    f32 = lambda a: np.ascontiguousarray(np.asarray(a, np.float32))

    d = {}
    d["vT"] = bf(np.stack([inp["v_r"].T, inp["v_l"].T]))            # [2,20,512]
    d["wihT0"] = bf(np.stack(
        [_pad_reorder_rows(inp["Wih_l0f"]).T, _pad_reorder_rows(inp["Wih_l0b"]).T]))  # [2,20,1024]
    d["wihT1"] = bf(np.stack(
        [_pad_cols_500(_pad_reorder_rows(inp["Wih_l1f"])).T,
         _pad_cols_500(_pad_reorder_rows(inp["Wih_l1b"])).T]))      # [2,512,1024]

    whh = []
    for l in ("l0", "l1"):
        for dd in ("f", "b"):
            w = _pad_reorder_rows(inp[f"Whh_{l}{dd}"])              # [1024, 250]
            w = np.concatenate([w, np.zeros((G4, 6), w.dtype)], axis=1)  # [1024,256]
            whh.append(w.T)                                          # [256,1024]
    d["whhT"] = bf(np.stack(whh).reshape(2, 2, HP, G4))

    bias = []
    for l in ("l0", "l1"):
        for dd in ("f", "b"):
            b = _pad_reorder_rows(inp[f"bih_{l}{dd}"] + inp[f"bhh_{l}{dd}"])
            bias.append(_chunk_bias(b))
    d["biasg"] = f32(np.stack(bias).reshape(2, 2, 128, 8))

    d["w1T"] = bf(_pad_cols_500(inp["W1"]).T)                        # [512,1024]
    d["b1c"] = f32(_chunk_bias(inp["b1"]))                           # [128,8]
    d["w2T"] = bf(inp["W2"].T)                                       # [1024,512]
    d["b2c"] = f32(_chunk_bias(inp["b2"]))                           # [128,4]
    d["w3aT"] = bf(inp["W3"][:, :H2].T)                              # [512,512]
    d["w3bT"] = bf(inp["W3"][:, H2:].T)                              # [512,512]
    d["b3c"] = f32(_chunk_bias(inp["b3"]))                           # [128,4]

    wout = np.asarray(inp["Wout"], np.float32)                       # [2,512]
    d["wdiffc"] = bf(_chunk_bias(wout[1] - wout[0]))                 # [128,4]
    db = float(inp["bout"][1] - inp["bout"][0])
    sfx = np.zeros((128, 4), np.float32)
    sfx[:, 0] = db
    sfx[:, 1] = -db
    sfx[:, 2] = -1.0
    d["sfx"] = sfx
    return d, db


# ----------------------------------------------------------------------------
# Device program
# ----------------------------------------------------------------------------

def _build_program(db):
    nc = bacc.Bacc("TRN2", target_bir_lowering=False, debug=False)

    d_vT = nc.dram_tensor("vT", [2, DIN, T], BF16, kind="ExternalInput")
    d_wihT0 = nc.dram_tensor("wihT0", [2, DIN, G4], BF16, kind="ExternalInput")
    d_wihT1 = nc.dram_tensor("wihT1", [2, 512, G4], BF16, kind="ExternalInput")
    d_whhT = nc.dram_tensor("whhT", [2, 2, HP, G4], BF16, kind="ExternalInput")
    d_biasg = nc.dram_tensor("biasg", [2, 2, 128, 8], F32, kind="ExternalInput")
    d_w1T = nc.dram_tensor("w1T", [512, H1], BF16, kind="ExternalInput")
    d_b1c = nc.dram_tensor("b1c", [128, 8], F32, kind="ExternalInput")
    d_w2T = nc.dram_tensor("w2T", [H1, H2], BF16, kind="ExternalInput")
    d_b2c = nc.dram_tensor("b2c", [128, 4], F32, kind="ExternalInput")
    d_w3aT = nc.dram_tensor("w3aT", [H2, H3], BF16, kind="ExternalInput")
    d_w3bT = nc.dram_tensor("w3bT", [H2, H3], BF16, kind="ExternalInput")
    d_b3c = nc.dram_tensor("b3c", [128, 4], F32, kind="ExternalInput")
    d_wdiffc = nc.dram_tensor("wdiffc", [128, 4], BF16, kind="ExternalInput")
    d_sfx = nc.dram_tensor("sfx", [128, 4], F32, kind="ExternalInput")
    d_pidv = nc.dram_tensor("pidv", [1, 1], mybir.dt.uint32, kind="ExternalInput")
    d_out = nc.dram_tensor("out", [RPC * T, RRI], F32, kind="ExternalOutput")

    with tile.TileContext(nc) as tc, ExitStack() as ctx:
        wts = ctx.enter_context(tc.tile_pool(name="wts", bufs=1))
        st = ctx.enter_context(tc.tile_pool(name="st", bufs=1))
        work = ctx.enter_context(tc.tile_pool(name="work", bufs=6))
        h3p = ctx.enter_context(tc.tile_pool(name="h3p", bufs=3))
        outp = ctx.enter_context(tc.tile_pool(name="outp", bufs=4))

        # ------------------------- load weights -------------------------
        whhT_sb = wts.tile([128, 2 * 2 * 2 * G4], BF16)
        whhT_v = whhT_sb.rearrange("p (l d k g) -> p l d k g", l=2, d=2, k=2)
        for l in range(2):
            for dd in range(2):
                nc.sync.dma_start(
                    whhT_v[:, l, dd, :, :],
                    d_whhT.ap()[l, dd].rearrange("(k p) g -> p k g", p=128))

        wihT0_sb = wts.tile([DIN, 2 * G4], BF16)
        wihT0_v = wihT0_sb.rearrange("p (d g) -> p d g", d=2)
        nc.sync.dma_start(wihT0_v[:, :, :], d_wihT0.ap().rearrange("d p g -> p d g"))

        wihT1_sb = wts.tile([128, 2 * 4 * G4], BF16)
        wihT1_v = wihT1_sb.rearrange("p (d k g) -> p d k g", d=2, k=4)
        for dd in range(2):
            nc.sync.dma_start(
                wihT1_v[:, dd, :, :],
                d_wihT1.ap()[dd].rearrange("(k p) g -> p k g", p=128))

        vT_sb = wts.tile([DIN, 2 * T], BF16)
        vT_v = vT_sb.rearrange("p (s t) -> p s t", s=2)
        nc.sync.dma_start(vT_v[:, :, :], d_vT.ap().rearrange("s p t -> p s t"))

        biasg_sb = wts.tile([128, 2 * 2 * 8], F32)
        biasg_v = biasg_sb.rearrange("p (l d m) -> p l d m", l=2, d=2)
        nc.sync.dma_start(biasg_v[:, :, :, :],
                          d_biasg.ap().rearrange("l d p m -> p l d m"))

        w1T_sb = wts.tile([128, 4 * H1], BF16)
        w1T_v = w1T_sb.rearrange("p (k g) -> p k g", k=4)
        nc.sync.dma_start(w1T_v[:, :, :],
                          d_w1T.ap().rearrange("(k p) g -> p k g", p=128))

        w2T_sb = wts.tile([128, 8 * H2], BF16)
        w2T_v = w2T_sb.rearrange("p (k g) -> p k g", k=8)
        nc.sync.dma_start(w2T_v[:, :, :],
                          d_w2T.ap().rearrange("(k p) g -> p k g", p=128))

        w3aT_sb = wts.tile([128, 4 * H3], BF16)
        w3aT_v = w3aT_sb.rearrange("p (k g) -> p k g", k=4)
        nc.sync.dma_start(w3aT_v[:, :, :],
                          d_w3aT.ap().rearrange("(k p) g -> p k g", p=128))

        w3bT_sb = wts.tile([128, 4 * H3], BF16)
        w3bT_v = w3bT_sb.rearrange("p (k g) -> p k g", k=4)
        nc.sync.dma_start(w3bT_v[:, :, :],
                          d_w3bT.ap().rearrange("(k p) g -> p k g", p=128))

        b1c_sb = wts.tile([128, 8], F32)
        nc.sync.dma_start(b1c_sb[:, :], d_b1c.ap())
        b2c_sb = wts.tile([128, 4], F32)
        nc.sync.dma_start(b2c_sb[:, :], d_b2c.ap())
        b3c_sb = wts.tile([128, 4], F32)
        nc.sync.dma_start(b3c_sb[:, :], d_b3c.ap())
        wdiffc_sb = wts.tile([128, 4], BF16)
        nc.sync.dma_start(wdiffc_sb[:, :], d_wdiffc.ap())
        sfx_sb = wts.tile([128, 4], F32)
        nc.sync.dma_start(sfx_sb[:, :], d_sfx.ap())
        pidv_sb = wts.tile([1, 1], mybir.dt.uint32)
        nc.sync.dma_start(pidv_sb[:, :], d_pidv.ap())

        # ------------------------- state buffers -------------------------
        # gx: cols (t, m, d, s) f32 -- reused by both layers
        gx_sb = st.tile([128, T * 32], F32)
        gx_v = gx_sb.rearrange("p (t m d s) -> p t m d s", t=T, m=8, d=2)
        # hist: bf16 h; per (layer, dir): cols (t, k, s)
        histF = [st.tile([128, T * 4], BF16, name=f"hF{l}") for l in range(2)]
        histB = [st.tile([128, T * 4], BF16, name=f"hB{l}") for l in range(2)]
        histF_v = [h.rearrange("p (t k s) -> p t k s", t=T, k=2) for h in histF]
        histB_v = [h.rearrange("p (t k s) -> p t k s", t=T, k=2) for h in histB]

        a1l_sb = st.tile([128, T * 8], BF16)
        a1l_v = a1l_sb.rearrange("p (t m) -> p t m", t=T)
        a1r_sb = st.tile([128, RPC * 8], BF16)
        a1r_v = a1r_sb.rearrange("p (t m) -> p t m", t=RPC)
        rl2l_sb = st.tile([128, T * 4], BF16)
        rl2l_v = rl2l_sb.rearrange("p (t m) -> p t m", t=T)
        rl2r_sb = st.tile([128, RPC * 4], BF16)
        rl2r_v = rl2r_sb.rearrange("p (t m) -> p t m", t=RPC)

        plT_sb = st.tile([128, 4 * T], BF16)      # cols (m, l)
        plT_v = plT_sb.rearrange("p (m l) -> p m l", m=4)
        prmy_sb = st.tile([128, 4 * RPC], F32)    # cols (m, i), includes b3
        prmy_v = prmy_sb.rearrange("p (m i) -> p m i", m=4)
        # this core's 64-step slices of layer-1 hist (both dirs)
        histrF = st.tile([128, RPC * 4], BF16)
        histrB = st.tile([128, RPC * 4], BF16)

        def recurrence(l, psg):
            """One layer; fwd/bwd merged per python step t."""
            hFv, hBv = histF_v[l], histB_v[l]
            c_prev = None
            for t in range(T):
                tF, tB = t, T - 1 - t
                if t > 0:
                    ps = psg.tile([128, 32], F32, name="ps_g")
                    for dd in range(2):
                        hv = hFv if dd == 0 else hBv
                        slot = tF - 1 if dd == 0 else tB + 1
                        for m in range(8):
                            for k in range(2):
                                nc.tensor.matmul(
                                    ps[:, 4 * m + 2 * dd:4 * m + 2 * dd + 2],
                                    whhT_v[:, l, dd, k, 128 * m:128 * (m + 1)],
                                    hv[:, slot, k, :],
                                    start=(k == 0), stop=(k == 1))
                    g_sb = work.tile([128, 32], F32, name="g_sb")
                    gv = g_sb.rearrange("p (m d s) -> p m d s", m=8, d=2)
                    ps_v = ps.rearrange("p (m d s) -> p m d s", m=8, d=2)
                    # bwd chain is at time tB: per-direction gx columns
                    nc.vector.tensor_tensor(
                        gv[:, :, 0, :], ps_v[:, :, 0, :], gx_v[:, tF, :, 0, :],
                        ALU.add)
                    nc.vector.tensor_tensor(
                        gv[:, :, 1, :], ps_v[:, :, 1, :], gx_v[:, tB, :, 1, :],
                        ALU.add)
                    g_in = g_sb
                else:
                    g_in = None

                sg = work.tile([128, 24], BF16, name="sg")
                tg = work.tile([128, 8], BF16, name="tg")
                if g_in is not None:
                    nc.scalar.activation(sg[:, :], g_in[:, 0:24], AF.Sigmoid)
                    nc.scalar.activation(tg[:, :], g_in[:, 24:32], AF.Tanh)
                else:
                    sg_d = sg.rearrange("p (m d s) -> p m d s", m=6, d=2)
                    tg_d = tg.rearrange("p (m d s) -> p m d s", m=2, d=2)
                    nc.scalar.activation(sg_d[:, :, 0, :], gx_v[:, tF, 0:6, 0, :],
                                         AF.Sigmoid)
                    nc.scalar.activation(sg_d[:, :, 1, :], gx_v[:, tB, 0:6, 1, :],
                                         AF.Sigmoid)
                    nc.scalar.activation(tg_d[:, :, 0, :], gx_v[:, tF, 6:8, 0, :],
                                         AF.Tanh)
                    nc.scalar.activation(tg_d[:, :, 1, :], gx_v[:, tB, 6:8, 1, :],
                                         AF.Tanh)

                t1 = work.tile([128, 8], F32, name="t1")
                nc.vector.tensor_tensor(t1[:, :], sg[:, 0:8], tg[:, :], ALU.mult)
                if t > 0:
                    t2 = work.tile([128, 8], F32, name="t2")
                    nc.vector.tensor_tensor(t2[:, :], sg[:, 8:16], c_prev[:, :],
                                            ALU.mult)
                    cn = work.tile([128, 8], F32, name="cn")
                    nc.vector.tensor_tensor(cn[:, :], t1[:, :], t2[:, :], ALU.add)
                else:
                    cn = t1
                c_prev = cn
                tc_t = work.tile([128, 8], BF16, name="tc_t")
                nc.scalar.activation(tc_t[:, :], cn[:, :], AF.Tanh)
                # cols of sg/tc blocks: (k, d, s).  h written per dir on GpSimd.
                sg_v = sg.rearrange("p (x k d s) -> p x k d s", x=3, k=2, d=2)
                tc_v = tc_t.rearrange("p (k d s) -> p k d s", k=2, d=2)
                nc.gpsimd.tensor_tensor(
                    hFv[:, tF, :, :], sg_v[:, 2, :, 0, :], tc_v[:, :, 0, :],
                    ALU.mult)
                nc.gpsimd.tensor_tensor(
                    hBv[:, tB, :, :], sg_v[:, 2, :, 1, :], tc_v[:, :, 1, :],
                    ALU.mult)

        with tc.tile_pool(name="psmm", bufs=4, space="PSUM") as psmm:
            # =============== layer-0 input projections (gx) ===============
            for dd in range(2):
                for s in range(2):
                    for m in range(8):
                        ps = psmm.tile([128, T], F32, name="ps_mm")
                        nc.tensor.matmul(
                            ps[:, :],
                            wihT0_v[:, dd, 128 * m:128 * (m + 1)],
                            vT_v[:, s, :], start=True, stop=True)
                        nc.scalar.activation(
                            gx_v[:, :, m, dd, s], ps[:, :],
                            AF.Identity, bias=biasg_v[:, 0, dd, m:m + 1])

        with tc.tile_pool(name="psg", bufs=4, space="PSUM") as psg:
            recurrence(0, psg)

        with tc.tile_pool(name="psmm", bufs=4, space="PSUM") as psmm:
            # ========== layer-1 input projections from hist0 ==========
            for dd in range(2):
                for s in range(2):
                    for m in range(8):
                        ps = psmm.tile([128, T], F32, name="ps_mm")
                        for k in range(4):
                            hsrc = histF[0] if k < 2 else histB[0]
                            kk = k % 2
                            rv = hsrc.rearrange("p (t k s) -> p k t s", t=T, k=2)
                            nc.tensor.matmul(
                                ps[:, :],
                                wihT1_v[:, dd, k, 128 * m:128 * (m + 1)],
                                rv[:, kk, :, s],
                                start=(k == 0), stop=(k == 3))
                        nc.scalar.activation(
                            gx_v[:, :, m, dd, s], ps[:, :],
                            AF.Identity, bias=biasg_v[:, 1, dd, m:m + 1])

        with tc.tile_pool(name="psg", bufs=4, space="PSUM") as psg:
            recurrence(1, psg)

        # pid register for the r-branch slice
        pid_reg = nc.vector.alloc_register("pid_reg")
        nc.vector.reg_load(pid_reg, pidv_sb[0:1, 0:1])
        pid = nc.vector.snap(pid_reg, donate=True, min_val=0, max_val=7)
        nc.vector.tensor_copy(histrF[:, :],
                              histF[1][:, bass.ds(pid * (RPC * 4), RPC * 4)])
        nc.vector.tensor_copy(histrB[:, :],
                              histB[1][:, bass.ds(pid * (RPC * 4), RPC * 4)])

        with tc.tile_pool(name="psmm", bufs=4, space="PSUM") as psmm:
            # ===================== branch MLP: a1 =====================
            # l-branch (seq 1) on full T; r-branch (seq 0) on own 64 rows.
            for m in range(8):
                ps = psmm.tile([128, T], F32, name="ps_mm")
                for k in range(4):
                    hsrc = histF[1] if k < 2 else histB[1]
                    kk = k % 2
                    rv = hsrc.rearrange("p (t k s) -> p k t s", t=T, k=2)
                    nc.tensor.matmul(
                        ps[:, :],
                        w1T_v[:, k, 128 * m:128 * (m + 1)],
                        rv[:, kk, :, 1],
                        start=(k == 0), stop=(k == 3))
                nc.scalar.activation(
                    a1l_v[:, :, m], ps[:, :], AF.Relu, bias=b1c_sb[:, m:m + 1])
            for m in range(8):
                ps = psmm.tile([128, RPC], F32, name="ps_r")
                for k in range(4):
                    hsrc = histrF if k < 2 else histrB
                    kk = k % 2
                    rv = hsrc.rearrange("p (t k s) -> p k t s", t=RPC, k=2)
                    nc.tensor.matmul(
                        ps[:, :],
                        w1T_v[:, k, 128 * m:128 * (m + 1)],
                        rv[:, kk, :, 0],
                        start=(k == 0), stop=(k == 3))
                nc.scalar.activation(
                    a1r_v[:, :, m], ps[:, :], AF.Relu, bias=b1c_sb[:, m:m + 1])

            # ===================== branch MLP: rl2 =====================
            for m in range(4):
                ps = psmm.tile([128, T], F32, name="ps_mm")
                for k in range(8):
                    nc.tensor.matmul(
                        ps[:, :], w2T_v[:, k, 128 * m:128 * (m + 1)],
                        a1l_v[:, :, k], start=(k == 0), stop=(k == 7))
                nc.scalar.activation(
                    rl2l_v[:, :, m], ps[:, :], AF.Relu, bias=b2c_sb[:, m:m + 1])
            for m in range(4):
                ps = psmm.tile([128, RPC], F32, name="ps_r")
                for k in range(8):
                    nc.tensor.matmul(
                        ps[:, :], w2T_v[:, k, 128 * m:128 * (m + 1)],
                        a1r_v[:, :, k], start=(k == 0), stop=(k == 7))
                nc.scalar.activation(
                    rl2r_v[:, :, m], ps[:, :], AF.Relu, bias=b2c_sb[:, m:m + 1])

            # pl = l2 @ W3b.T (bf16, cols (m,l));  prmy = own r2 @ W3a.T + b3
            for m in range(4):
                ps = psmm.tile([128, T], F32, name="ps_mm")
                for k in range(4):
                    nc.tensor.matmul(
                        ps[:, :], w3bT_v[:, k, 128 * m:128 * (m + 1)],
                        rl2l_v[:, :, k], start=(k == 0), stop=(k == 3))
                nc.scalar.activation(plT_v[:, m, :], ps[:, :], AF.Identity)
            for m in range(4):
                ps = psmm.tile([128, RPC], F32, name="ps_r")
                for k in range(4):
                    nc.tensor.matmul(
                        ps[:, :], w3aT_v[:, k, 128 * m:128 * (m + 1)],
                        rl2r_v[:, :, k], start=(k == 0), stop=(k == 3))
                nc.scalar.activation(
                    prmy_v[:, m, :], ps[:, :], AF.Identity,
                    bias=b3c_sb[:, m:m + 1])

        # ========================= pairwise stage =========================
        with tc.tile_pool(name="pslg", bufs=1, space="PSUM") as pslg:
            lgp = [pslg.tile([128, RPC], F32, name=f"lg{lb}") for lb in range(4)]

            for i in range(RPC):
                h3 = h3p.tile([128, 4 * H3], BF16, name="h3")
                h3_v = h3.rearrange("p (m l) -> p m l", m=4)
                ndve = 2 if (i % 2 == 0) else 3
                for m in range(4):
                    if m < ndve:
                        nc.vector.tensor_scalar(
                            h3_v[:, m, :], plT_v[:, m, :],
                            prmy_v[:, m, i:i + 1], 0.0, ALU.add, ALU.max)
                    else:
                        nc.scalar.activation(
                            h3_v[:, m, :], plT_v[:, m, :], AF.Relu,
                            bias=prmy_v[:, m, i:i + 1])
                for lb in range(4):
                    for m in range(4):
                        nc.tensor.matmul(
                            lgp[lb][:, i:i + 1],
                            h3_v[:, m, 128 * lb:128 * (lb + 1)],
                            wdiffc_sb[:, m:m + 1],
                            start=(m == 0), stop=(m == 3))

            # log_softmax over the 2 classes + output DMA.
            # out0 = ln sig(-(d+db)), out1 = ln sig(d+db)
            out_v = d_out.ap().rearrange("(r q l) k -> q l r k", q=4, l=128)
            sig_tiles = []
            for lb in range(4):
                lgs = outp.tile([128, RPC], F32, name="lgs")
                nc.vector.tensor_copy(lgs[:, :], lgp[lb][:, :])
                s0 = outp.tile([128, RPC], F32, name="s0")
                nc.scalar.activation(s0[:, :], lgs[:, :], AF.Sigmoid,
                                     bias=sfx_sb[:, 1:2], scale=sfx_sb[:, 2:3])
                s1 = outp.tile([128, RPC], F32, name="s1")
                nc.scalar.activation(s1[:, :], lgs[:, :], AF.Sigmoid,
                                     bias=sfx_sb[:, 0:1])
                sig_tiles.append((s0, s1))
            for lb in range(4):
                s0, s1 = sig_tiles[lb]
                osb = outp.tile([128, 2 * RPC], F32, name="osb")
                osb_v = osb.rearrange("p (r k) -> p r k", k=2)
                nc.scalar.activation(osb_v[:, :, 0], s0[:, :], AF.Ln)
                nc.scalar.activation(osb_v[:, :, 1], s1[:, :], AF.Ln)
                nc.sync.dma_start(out_v[lb], osb_v[:, :, :])

    nc.compile()
    return nc


_CACHE = {}


def kernel(**inputs):
    inputs = {k: np.asarray(v) for k, v in inputs.items()}
    d, db = _prep_inputs(inputs)

    key = round(db, 10)
    if key not in _CACHE:
        _CACHE[key] = _build_program(db)
    nc = _CACHE[key]

    in_maps = [dict(d, pidv=np.array([[c]], np.uint32)) for c in range(NCORES)]
    res = run_bass_kernel_spmd(nc, in_maps, core_ids=list(range(NCORES)))
    out = np.concatenate([res.results[c]["out"] for c in range(NCORES)], axis=0)
    return out.astype(np.float32)


if __name__ == "__main__":
    sys.path.insert(0, "/root/problem")
    import reference
    inp = {k: np.asarray(v) for k, v in reference.setup_inputs().items()}
    got = kernel(**inp)
    print("out shape", got.shape, got.dtype)
